# revision 1
# baseline (speedup 1.0000x reference)
"""Trainium2 Bass kernel for NCM/kNN retrieval (nn_NCM_30468497998426).

reference computation:
    mean-center support [C=1000,S=5,D=512] and queries [Q=5000,D=512] by the
    support mean, L2-normalize, sims = einsum('csd,qd->cqs'), max over shots,
    argmax over classes -> [Q] int32.

Sharding: queries split across 8 cores (625 each), support replicated.

v2 layout: support is HOST-transposed to [512, 5000] (pure layout change),
so the kernel needs NO PE transposes or PSUM->SBUF copybacks at all:
    - 4 big DMAs load support d-major chunks [128, 5000]; queries arrive
      host-transposed+zero-padded as [512, 640].
    - mean: chunked free-dim reduces (DVE tensor_reduce / ACT accum split)
      pipelined under the DMA -> mu_T [128,1] per d-chunk.
    - queries centered with per-partition mu_T, rounded to f32r.
    - per cs-chunk j (500 cols): ACT Square(x + bias=-mu) emits squared
      centered values (rounded to f32r), f32r ones-matmul column-sums them
      in PSUM -> |s-mu|^2 replicated across partitions; ACT Sqrt + DVE
      reciprocal -> inv_rep [128,500]; one fused scalar_tensor_tensor pass
      (x - mu)*inv_rep produces the normalized support chunk in f32r.
    - mains: 4 f32r accumulate-matmuls -> PSUM sims [128q, 500cs], DVE max
      over shots -> best[q, 100c]; final argmax via MAX8.
"""

import numpy as np

import concourse.bacc as bacc
import concourse.mybir as mybir
import concourse.tile as tile
from concourse.alu_op_type import AluOpType
from concourse.bass_utils import run_bass_kernel_spmd

F32 = mybir.dt.float32
F32R = mybir.dt.float32r
I32 = mybir.dt.int32
U32 = mybir.dt.uint32
BF16 = mybir.dt.bfloat16
AF = mybir.ActivationFunctionType

C, S, D = 1000, 5, 512
CS = C * S              # 5000 support rows
Q = 5000
NCORES = 8
QS = Q // NCORES        # 625 queries per core
QSP = 640               # padded to 5x128 (f32r wants even/128-wide tiles)
PW = 128                # queries per stationary tile
KC = D // 128           # 4 contraction chunks
QT = QSP // PW          # 5 query tiles
CSCH = 500              # cs per PSUM chunk
NJ = CS // CSCH         # 10 cs chunks
GPC = CSCH // S         # classes per chunk (100)
MR = 8                  # mean sub-reduces per d-chunk
MW = CS // MR           # 625 cols per mean sub-reduce


def build():
    nc = bacc.Bacc(None, target_bir_lowering=False)

    sup_t = nc.declare_dram_parameter("support_t", [D, CS], F32, isOutput=False)
    qry_t = nc.declare_dram_parameter("queries_t", [D, QSP], F32, isOutput=False)
    ones_cr = nc.declare_dram_parameter("ones_cr", [128, 128], F32, isOutput=False)
    out = nc.declare_dram_parameter("out", [QS, 1], I32, isOutput=True)

    def r(ap):
        return ap.bitcast(F32R)

    with tile.TileContext(nc) as tc:
        with (
            tc.tile_pool(name="const", bufs=1) as pconst,
            tc.tile_pool(name="stat", bufs=1) as pstat,
            tc.tile_pool(name="sраw", bufs=1) as praw,
            tc.tile_pool(name="st", bufs=3 * KC) as pst,
            tc.tile_pool(name="qt", bufs=1) as pqt,
            tc.tile_pool(name="qc", bufs=1) as pqc,
            tc.tile_pool(name="sq", bufs=3) as psq,
            tc.tile_pool(name="nrm", bufs=2) as pnrm,
            tc.tile_pool(name="best", bufs=1) as pbest,
            tc.tile_pool(name="res", bufs=2) as pres,
            tc.tile_pool(name="n2psum", bufs=2, space="PSUM") as pn2,
            tc.tile_pool(name="mmpsum", bufs=6, space="PSUM") as pmm,
        ):
            ones_sb = pconst.tile([128, 128], F32, tag="ones")
            nc.sync.dma_start(ones_sb[:], ones_cr[:])

            # ---- loads: queries first (small), then 4 support d-chunks
            qt_tiles = []
            with nc.named_scope("load_q"):
                for k in range(KC):
                    qt_ = pqt.tile([128, QSP], F32, name=f"qt{k}", tag=f"qt{k}")
                    nc.sync.dma_start(qt_[:], qry_t[k * 128:(k + 1) * 128, :])
                    qt_tiles.append(qt_)
            st_raw = []
            with nc.named_scope("load_s"):
                for k in range(KC):
                    st_ = praw.tile([128, CS], F32, name=f"sraw{k}", tag=f"sraw{k}")
                    nc.sync.dma_start(st_[:], sup_t[k * 128:(k + 1) * 128, :])
                    st_raw.append(st_)

            # ---- mean: free-dim reduces pipelined under the DMA
            with nc.named_scope("mean"):
                msubs = [pstat.tile([128, MR], F32, tag=f"msub{k}",
                                    name=f"msub{k}") for k in range(KC)]
                for k in range(KC):
                    for m in range(MR):
                        sl = st_raw[k][:, m * MW:(m + 1) * MW]
                        col = msubs[k][:, m:m + 1]
                        if m % 2 == 0:
                            nc.vector.tensor_reduce(
                                out=col, in_=sl, axis=mybir.AxisListType.X,
                                op=AluOpType.add)
                        else:
                            dump = psq.tile([128, MW], F32, tag="mdump")
                            nc.scalar.activation(dump[:], sl, AF.Copy,
                                                 accum_out=col)
                mu_t, nmu_t = [], []
                for k in range(KC):
                    acc = pstat.tile([128, 1], F32, tag=f"macc{k}")
                    nc.vector.tensor_reduce(
                        out=acc[:], in_=msubs[k][:],
                        axis=mybir.AxisListType.X, op=AluOpType.add)
                    mu = pstat.tile([128, 1], F32, tag=f"mu{k}", name=f"mu{k}")
                    nc.vector.tensor_scalar_mul(mu[:], acc[:], 1.0 / CS)
                    nmu = pstat.tile([128, 1], F32, tag=f"nmu{k}",
                                     name=f"nmu{k}")
                    nc.vector.tensor_scalar_mul(nmu[:], acc[:], -1.0 / CS)
                    mu_t.append(mu)
                    nmu_t.append(nmu)

            # ---- query side: center exactly, split into bf16 hi/lo
            qhi_tiles, qlo_tiles = [], []
            with nc.named_scope("qside"):
                for k in range(KC):
                    qc = pqc.tile([128, QSP], F32, name=f"qc{k}", tag=f"qc{k}")
                    nc.vector.tensor_scalar_sub(qc[:], qt_tiles[k][:],
                                                mu_t[k][:])
                    qhi = pqc.tile([128, QSP], BF16, name=f"qhi{k}",
                                   tag=f"qhi{k}")
                    nc.scalar.copy(qhi[:], qc[:])
                    qlo = pqc.tile([128, QSP], BF16, name=f"qlo{k}",
                                   tag=f"qlo{k}")
                    nc.vector.tensor_sub(qlo[:], qc[:], qhi[:])
                    qhi_tiles.append(qhi)
                    qlo_tiles.append(qlo)

            # ---- pipelined per cs-chunk: norms -> normalize -> matmul
            best_tiles = [pbest.tile([PW, C], F32, name=f"best{i}", tag=f"best{i}")
                          for i in range(QT)]
            def emit_prep(j):
                cs0 = j * CSCH
                with nc.named_scope(f"prep{j}"):
                    n2_ps = pn2.tile([128, CSCH], F32, tag="n2",
                                     name=f"n2_{j}")
                    for k in range(KC):
                        sqj = psq.tile([128, CSCH], F32, tag="sq",
                                       name=f"sq{k}_{j}")
                        nc.scalar.activation(
                            sqj[:], st_raw[k][:, cs0:cs0 + CSCH],
                            AF.Square, bias=nmu_t[k][:])
                        nc.tensor.matmul(n2_ps[:], ones_sb[:], sqj[:],
                                         start=(k == 0), stop=(k == KC - 1))
                    nrm = pnrm.tile([128, CSCH], F32, tag="nrm",
                                    name=f"nrm{j}")
                    nc.scalar.activation(nrm[:], n2_ps[:], AF.Sqrt)
                    inv = pnrm.tile([128, CSCH], F32, tag="inv",
                                    name=f"inv{j}")
                    rscr = pnrm.tile([128, CSCH], F32, tag="rscr",
                                     name=f"rscr{j}")
                    nc.vector.reciprocal_approx_accurate(inv[:], nrm[:],
                                                         rscr[:])
                    shi, slo = [], []
                    for k in range(KC):
                        st_ = pst.tile([128, CSCH], F32, name=f"st{k}_{j}",
                                       tag="st")
                        nc.vector.scalar_tensor_tensor(
                            out=st_[:], in0=st_raw[k][:, cs0:cs0 + CSCH],
                            scalar=mu_t[k][:], in1=inv[:],
                            op0=AluOpType.subtract, op1=AluOpType.mult)
                        hi = pst.tile([128, CSCH], BF16, name=f"shi{k}_{j}",
                                      tag="shi")
                        nc.scalar.copy(hi[:], st_[:])
                        lo = pst.tile([128, CSCH], BF16, name=f"slo{k}_{j}",
                                      tag="slo")
                        if k < 2:
                            nc.vector.tensor_sub(lo[:], st_[:], hi[:])
                        else:
                            nc.gpsimd.tensor_sub(lo[:], st_[:], hi[:])
                        shi.append(hi)
                        slo.append(lo)
                return shi, slo

            def emit_mm(j, shi, slo):
                with nc.named_scope(f"mm{j}"):
                    ps_list = [pmm.tile([PW, CSCH], F32, tag="sims",
                                        name=f"ps{i}_{j}")
                               for i in range(QT)]
                    nmm = 3 * KC
                    passes = [(k, hs, ss)
                              for k in range(KC)
                              for hs, ss in ((qhi_tiles, shi),
                                             (qhi_tiles, slo),
                                             (qlo_tiles, shi))]
                    if j < NJ - 1:
                        for pidx, (k, hs, ss) in enumerate(passes):
                            for i in range(QT):
                                nc.tensor.matmul(
                                    ps_list[i][:],
                                    hs[k][:, i * PW:(i + 1) * PW],
                                    ss[k][:],
                                    start=(pidx == 0), stop=(pidx == nmm - 1),
                                )
                        for i in range(QT):
                            nc.vector.tensor_reduce(
                                out=best_tiles[i][:, j * GPC:(j + 1) * GPC],
                                in_=ps_list[i][:].rearrange(
                                    "p (c s) -> p c s", s=S),
                                axis=mybir.AxisListType.X, op=AluOpType.max,
                            )
                    else:
                        # last chunk: i-outer so tile i's reduce drains while
                        # tile i+1 is still on the PE (shrinks the tail)
                        for i in range(QT):
                            for pidx, (k, hs, ss) in enumerate(passes):
                                nc.tensor.matmul(
                                    ps_list[i][:],
                                    hs[k][:, i * PW:(i + 1) * PW],
                                    ss[k][:],
                                    start=(pidx == 0), stop=(pidx == nmm - 1),
                                )
                            nc.vector.tensor_reduce(
                                out=best_tiles[i][:, j * GPC:(j + 1) * GPC],
                                in_=ps_list[i][:].rearrange(
                                    "p (c s) -> p c s", s=S),
                                axis=mybir.AxisListType.X, op=AluOpType.max,
                            )

            # software pipeline: prep(j+1) is emitted BEFORE mains(j) so the
            # next chunk's norms/normalize run ahead on every engine queue
            mxa, ixa = [], []
            pend = [emit_prep(0), emit_prep(1)]
            for j in range(NJ):
                cur = pend.pop(0)
                if j + 2 < NJ:
                    pend.append(emit_prep(j + 2))
                emit_mm(j, *cur)
                if j == NJ // 2 - 1:
                    with nc.named_scope("argmaxA"):
                        for i in range(QT):
                            ma = pres.tile([PW, 8], F32, tag=f"mxa{i}",
                                           name=f"mxa{i}", bufs=1)
                            ia = pres.tile([PW, 8], U32, tag=f"ixa{i}",
                                           name=f"ixa{i}", bufs=1)
                            nc.vector.max_with_indices(
                                ma[:], ia[:], best_tiles[i][:, 0:C // 2])
                            mxa.append(ma)
                            ixa.append(ia)

            # ---- argmax half B + combine with half A
            with nc.named_scope("argmaxB"):
                for i in range(QT):
                    valid = min(PW, QS - i * PW)
                    mxb = pres.tile([PW, 8], F32, tag="mxb")
                    ixb = pres.tile([PW, 8], U32, tag="ixb")
                    nc.vector.max_with_indices(mxb[:], ixb[:],
                                               best_tiles[i][:, C // 2:C])
                    ib5 = pres.tile([PW, 1], U32, tag="ib5")
                    nc.vector.tensor_scalar_add(ib5[:], ixb[:, 0:1], C // 2)
                    sel = pres.tile([PW, 1], U32, tag="sel")
                    nc.vector.tensor_tensor(
                        out=sel[:], in0=mxb[:, 0:1], in1=mxa[i][:, 0:1],
                        op=AluOpType.is_gt)
                    iu = pres.tile([PW, 1], U32, tag="iu")
                    nc.vector.select(iu[:], sel[:], ib5[:], ixa[i][:, 0:1])
                    ii = pres.tile([PW, 1], I32, tag="ii")
                    nc.vector.tensor_copy(ii[:], iu[:])
                    nc.sync.dma_start(out[i * PW:i * PW + valid, :],
                                      ii[0:valid, :])

    nc.finalize()
    return nc


def _host_inputs(support_features, query_features):
    sup = np.asarray(support_features, dtype=np.float32).reshape(CS, D)
    sup_t = np.ascontiguousarray(sup.T)
    qf = np.asarray(query_features, dtype=np.float32)
    ones_cr = np.ones((128, 128), dtype=np.float32)
    in_maps = []
    for c in range(NCORES):
        qslab = np.zeros((QSP, D), dtype=np.float32)
        qslab[:QS] = qf[c * QS:(c + 1) * QS]
        in_maps.append({
            "support_t": sup_t,
            "queries_t": np.ascontiguousarray(qslab.T),
            "ones_cr": ones_cr,
        })
    return in_maps


def run(support_features, query_features, trace=False, **trace_kwargs):
    nc = build()
    in_maps = _host_inputs(support_features, query_features)
    res = run_bass_kernel_spmd(nc, in_maps, list(range(NCORES)),
                               trace=trace, **trace_kwargs)
    outs = [np.asarray(r["out"]).reshape(QS) for r in res.results]
    return np.concatenate(outs).astype(np.int32), res


def kernel(support_features, query_features, use_cosine=None, **_ignored):
    # use_cosine does not change the result: with L2-normalized vectors the
    # euclidean argmin equals the cosine argmax (monotone map), so one kernel
    # serves both branches.
    out, _ = run(support_features, query_features, trace=False)
    return out



# revision 14
# speedup vs baseline: 1.5274x; 1.5274x over previous
"""Trainium2 Bass kernel for NCM/kNN retrieval (nn_NCM_30468497998426).

reference computation:
    mean-center support [C=1000,S=5,D=512] and queries [Q=5000,D=512] by the
    support mean, L2-normalize, sims = einsum('csd,qd->cqs'), max over shots,
    argmax over classes -> [Q] int32.

Sharding: queries split across 8 cores (625 each), support replicated.

v3: single-pass f32r matmuls (FP22-truncated inputs stream at bf16 rate on
the PE) replace the old 3-pass bf16 hi/lo scheme; |q| normalization is
skipped entirely (monotone per query).
    - support [512,5000] + queries [512,640] arrive host-transposed; support
      DMA is split into [128,500] sub-tiles so the mean reduces, the
      Square(x-mu) passes and the f32r ones-matmul norm accumulations all
      pipeline under the DMA (also keeps the PE warm for HAM).
    - per cs-chunk j: n2 = ones_r.T @ Square(x+(-mu)) in PSUM; ACT Sqrt;
      DVE reciprocal (approx+NR); normalize split ACT-center (Copy w/ bias)
      + GpSimd multiply -> st_ f32r.
    - mains: 4 f32r accumulate-matmuls [128q x 500cs] per query tile; DVE
      max over shots -> best[q, 1000c].
    - argmax staged: classes 0:500 at j=4, 500:900 at j=8, 900:1000 plus
      combine + store interleaved per query tile inside the last chunk.
"""

import numpy as np

import concourse.bacc as bacc
import concourse.mybir as mybir
import concourse.tile as tile
from concourse.alu_op_type import AluOpType
from concourse.bass_utils import run_bass_kernel_spmd

F32 = mybir.dt.float32
F32R = mybir.dt.float32r
I32 = mybir.dt.int32
U32 = mybir.dt.uint32
AF = mybir.ActivationFunctionType

C, S, D = 1000, 5, 512
CS = C * S              # 5000 support rows
Q = 5000
NCORES = 8
QS = Q // NCORES        # 625 queries per core
QSP = 640               # padded to 5x128
PW = 128                # queries per stationary tile
KC = D // 128           # 4 contraction chunks
QT = QSP // PW          # 5 query tiles
CSCH = 500              # cs per PSUM chunk
NJ = CS // CSCH         # 10 cs chunks
GPC = CSCH // S         # classes per chunk (100)
NHEAD = 3               # cs-chunks whose norm-stats pre-run under the DMA


def build():
    nc = bacc.Bacc(None, target_bir_lowering=False)

    sup_t = nc.declare_dram_parameter("support_t", [D, CS], F32, isOutput=False)
    qry_t = nc.declare_dram_parameter("queries_t", [D, QSP], F32, isOutput=False)
    ones_cr = nc.declare_dram_parameter("ones_cr", [128, 128], F32, isOutput=False)
    out = nc.declare_dram_parameter("out", [QS, 1], I32, isOutput=True)

    with tile.TileContext(nc) as tc:
        with (
            tc.tile_pool(name="const", bufs=1) as pconst,
            tc.tile_pool(name="stat", bufs=1) as pstat,
            tc.tile_pool(name="sraw", bufs=1) as praw,
            tc.tile_pool(name="ctr", bufs=6) as pctr,
            tc.tile_pool(name="st", bufs=2 * KC) as pst,
            tc.tile_pool(name="qt", bufs=1) as pqt,
            tc.tile_pool(name="qc", bufs=1) as pqc,
            tc.tile_pool(name="sq", bufs=6) as psq,
            tc.tile_pool(name="nrm", bufs=2) as pnrm,
            tc.tile_pool(name="best", bufs=1) as pbest,
            tc.tile_pool(name="res", bufs=2) as pres,
            tc.tile_pool(name="n2psum", bufs=3, space="PSUM") as pn2,
            tc.tile_pool(name="mmpsum", bufs=5, space="PSUM") as pmm,
        ):
            ones_sb = pconst.tile([128, 128], F32, tag="ones")
            nc.sync.dma_start(ones_sb[:], ones_cr[:])
            ones_r = pconst.tile([128, 128], F32R, tag="ones_r")
            nc.vector.tensor_copy(ones_r[:], ones_sb[:])

            # ---- queries first (small), then support as [128,500] sub-DMAs
            qt_tiles = []
            with nc.named_scope("load_q"):
                for k in range(KC):
                    qt_ = pqt.tile([128, QSP], F32, name=f"qt{k}", tag=f"qt{k}")
                    nc.sync.dma_start(qt_[:], qry_t[k * 128:(k + 1) * 128, :])
                    qt_tiles.append(qt_)
            st_raw = [praw.tile([128, CS], F32, name=f"sraw{k}", tag=f"sraw{k}")
                      for k in range(KC)]
            with nc.named_scope("load_s"):
                for k in range(KC):
                    for m in range(NJ):
                        nc.sync.dma_start(
                            st_raw[k][:, m * CSCH:(m + 1) * CSCH],
                            sup_t[k * 128:(k + 1) * 128,
                                  m * CSCH:(m + 1) * CSCH])

            n2_tiles = {}
            qc_tiles = []

            def emit_n2pass(k, j):
                # Square(x - mu) then f32r ones-matmul accumulate into n2(j).
                sqj = psq.tile([128, CSCH], F32R, tag="sq", name=f"sq{k}_{j}")
                nc.scalar.activation(sqj[:], st_raw[k][:, j * CSCH:(j + 1) * CSCH],
                                     AF.Square, bias=nmu_t[k][:])
                nc.tensor.matmul(n2_tiles[j][:], ones_r[:], sqj[:],
                                 start=(k == 0), stop=(k == KC - 1))

            # ---- mean, k-pipelined: reduces fire as each sub-DMA lands;
            # squares/norm-matmuls for the first NHEAD chunks follow each mu_k.
            mu_t, nmu_t = [], []
            with nc.named_scope("mean"):
                for k in range(KC):
                    msub = pstat.tile([128, NJ], F32, tag=f"msub{k}",
                                      name=f"msub{k}")
                    for m in range(NJ):
                        sl = st_raw[k][:, m * CSCH:(m + 1) * CSCH]
                        col = msub[:, m:m + 1]
                        if m % 4 == 3:
                            dump = psq.tile([128, CSCH], F32, tag="mdump")
                            nc.scalar.activation(dump[:], sl, AF.Copy,
                                                 accum_out=col)
                        else:
                            nc.vector.tensor_reduce(
                                out=col, in_=sl, axis=mybir.AxisListType.X,
                                op=AluOpType.add)
                    # ACT-side copy: strict FIFO on ACT orders it after the
                    # accum_out column writes; its tracked output write makes
                    # the cross-engine reduce below race-free.
                    msubc = pstat.tile([128, NJ], F32, tag=f"msubc{k}",
                                       name=f"msubc{k}")
                    nc.scalar.activation(msubc[:], msub[:], AF.Copy)
                    acc = pstat.tile([128, 1], F32, tag=f"macc{k}")
                    nc.vector.tensor_reduce(
                        out=acc[:], in_=msubc[:], axis=mybir.AxisListType.X,
                        op=AluOpType.add)
                    mu = pstat.tile([128, 1], F32, tag=f"mu{k}", name=f"mu{k}")
                    nc.vector.tensor_scalar_mul(mu[:], acc[:], 1.0 / CS)
                    nmu = pstat.tile([128, 1], F32, tag=f"nmu{k}",
                                     name=f"nmu{k}")
                    nc.vector.tensor_scalar_mul(nmu[:], acc[:], -1.0 / CS)
                    mu_t.append(mu)
                    nmu_t.append(nmu)
                    # center queries for this d-chunk (ACT: Copy with bias)
                    qc = pqc.tile([128, QSP], F32R, name=f"qc{k}",
                                  tag=f"qc{k}")
                    nc.scalar.activation(qc[:], qt_tiles[k][:], AF.Identity,
                                         bias=nmu[:])
                    qc_tiles.append(qc)
                    # pre-run the first chunks' norm stats under the DMA
                    for j in range(NHEAD):
                        if k == 0:
                            n2_tiles[j] = pn2.tile([128, CSCH], F32, tag="n2",
                                                   name=f"n2_{j}")
                        emit_n2pass(k, j)

            def emit_prep(j):
                with nc.named_scope(f"prep{j}"):
                    if j >= NHEAD:
                        n2_tiles[j] = pn2.tile([128, CSCH], F32, tag="n2",
                                               name=f"n2_{j}")
                        for k in range(KC):
                            emit_n2pass(k, j)
                    nrm = pnrm.tile([128, CSCH], F32, tag="nrm",
                                    name=f"nrm{j}")
                    nc.scalar.activation(nrm[:], n2_tiles[j][:], AF.Sqrt)
                    inv = pnrm.tile([128, CSCH], F32, tag="inv",
                                    name=f"inv{j}")
                    rscr = pnrm.tile([128, CSCH], F32, tag="rscr",
                                     name=f"rscr{j}")
                    nc.vector.reciprocal_approx_accurate(inv[:], nrm[:],
                                                         rscr[:])
                    sts = []
                    for k in range(KC):
                        ctr = pctr.tile([128, CSCH], F32, tag="ctr",
                                        name=f"ctr{k}_{j}")
                        nc.scalar.activation(
                            ctr[:], st_raw[k][:, j * CSCH:(j + 1) * CSCH],
                            AF.Identity, bias=nmu_t[k][:])
                        st_ = pst.tile([128, CSCH], F32R, name=f"st{k}_{j}",
                                       tag="st")
                        nc.gpsimd.tensor_tensor(out=st_[:], in0=ctr[:],
                                                in1=inv[:],
                                                op=AluOpType.mult)
                        sts.append(st_)
                return sts

            best_tiles = [pbest.tile([PW, C], F32, name=f"best{i}",
                                     tag=f"best{i}") for i in range(QT)]
            mxa, ixa = [], []

            def emit_argmax_stage(lo, hi, stage):
                # running argmax over class columns [lo:hi), merged into mxa/ixa
                for i in range(QT):
                    mx = pres.tile([PW, 8], F32, tag=f"mx{stage}{i}",
                                   name=f"mx{stage}{i}", bufs=1)
                    ix = pres.tile([PW, 8], U32, tag=f"ix{stage}{i}",
                                   name=f"ix{stage}{i}", bufs=1)
                    nc.vector.max_with_indices(mx[:], ix[:],
                                               best_tiles[i][:, lo:hi])
                    if stage == 0:
                        mxa.append(mx)
                        ixa.append(ix)
                    else:
                        ixo = pres.tile([PW, 1], U32, tag="ixo", bufs=2)
                        nc.vector.tensor_scalar_add(ixo[:], ix[:, 0:1], lo)
                        sel = pres.tile([PW, 1], U32, tag="sel", bufs=2)
                        nc.vector.tensor_tensor(
                            out=sel[:], in0=mx[:, 0:1], in1=mxa[i][:, 0:1],
                            op=AluOpType.is_gt)
                        ixm = pres.tile([PW, 1], U32, tag=f"ixm{i}", bufs=1,
                                        name=f"ixm{i}")
                        nc.vector.select(ixm[:], sel[:], ixo[:],
                                         ixa[i][:, 0:1])
                        mxm = pres.tile([PW, 1], F32, tag=f"mxm{i}", bufs=1,
                                        name=f"mxm{i}")
                        nc.vector.select(mxm[:], sel[:], mx[:, 0:1],
                                         mxa[i][:, 0:1])
                        mxa[i], ixa[i] = mxm, ixm

            def emit_mm(j, sts):
                with nc.named_scope(f"mm{j}"):
                    ps_list = [pmm.tile([PW, CSCH], F32, tag="sims",
                                        name=f"ps{i}_{j}")
                               for i in range(QT)]
                    if j < NJ - 1:
                        for k in range(KC):
                            for i in range(QT):
                                nc.tensor.matmul(
                                    ps_list[i][:],
                                    qc_tiles[k][:, i * PW:(i + 1) * PW],
                                    sts[k][:],
                                    start=(k == 0), stop=(k == KC - 1),
                                )
                        for i in range(QT):
                            nc.vector.tensor_reduce(
                                out=best_tiles[i][:, j * GPC:(j + 1) * GPC],
                                in_=ps_list[i][:].rearrange(
                                    "p (c s) -> p c s", s=S),
                                axis=mybir.AxisListType.X, op=AluOpType.max,
                            )
                    else:
                        # last chunk: per-tile matmul -> reduce -> final
                        # argmax slice -> combine -> store, all interleaved
                        for i in range(QT):
                            valid = min(PW, QS - i * PW)
                            for k in range(KC):
                                nc.tensor.matmul(
                                    ps_list[i][:],
                                    qc_tiles[k][:, i * PW:(i + 1) * PW],
                                    sts[k][:],
                                    start=(k == 0), stop=(k == KC - 1),
                                )
                            nc.vector.tensor_reduce(
                                out=best_tiles[i][:, j * GPC:(j + 1) * GPC],
                                in_=ps_list[i][:].rearrange(
                                    "p (c s) -> p c s", s=S),
                                axis=mybir.AxisListType.X, op=AluOpType.max,
                            )
                            mxc = pres.tile([PW, 8], F32, tag="mxc", bufs=2)
                            ixc = pres.tile([PW, 8], U32, tag="ixc", bufs=2)
                            nc.vector.max_with_indices(
                                mxc[:], ixc[:], best_tiles[i][:, 900:C])
                            ic9 = pres.tile([PW, 1], U32, tag="ic9", bufs=2)
                            nc.vector.tensor_scalar_add(ic9[:], ixc[:, 0:1],
                                                        900)
                            sel = pres.tile([PW, 1], U32, tag="self", bufs=2)
                            nc.vector.tensor_tensor(
                                out=sel[:], in0=mxc[:, 0:1],
                                in1=mxa[i][:, 0:1], op=AluOpType.is_gt)
                            iu = pres.tile([PW, 1], U32, tag="iu", bufs=2)
                            nc.vector.select(iu[:], sel[:], ic9[:],
                                             ixa[i][:, 0:1])
                            ii = pres.tile([PW, 1], I32, tag="ii", bufs=2)
                            nc.vector.tensor_copy(ii[:], iu[:])
                            nc.sync.dma_start(out[i * PW:i * PW + valid, :],
                                              ii[0:valid, :])

            # software pipeline: prep(j+2) emitted before mains(j)
            pend = [emit_prep(0), emit_prep(1)]
            for j in range(NJ):
                cur = pend.pop(0)
                if j + 2 < NJ:
                    pend.append(emit_prep(j + 2))
                emit_mm(j, cur)
                if j == 4:
                    with nc.named_scope("argmaxA"):
                        emit_argmax_stage(0, 500, 0)
                if j == 8:
                    with nc.named_scope("argmaxB"):
                        emit_argmax_stage(500, 900, 1)

    nc.finalize()
    return nc


def _host_inputs(support_features, query_features):
    sup = np.asarray(support_features, dtype=np.float32).reshape(CS, D)
    sup_t = np.ascontiguousarray(sup.T)
    qf = np.asarray(query_features, dtype=np.float32)
    ones_cr = np.ones((128, 128), dtype=np.float32)
    in_maps = []
    for c in range(NCORES):
        qslab = np.zeros((QSP, D), dtype=np.float32)
        qslab[:QS] = qf[c * QS:(c + 1) * QS]
        in_maps.append({
            "support_t": sup_t,
            "queries_t": np.ascontiguousarray(qslab.T),
            "ones_cr": ones_cr,
        })
    return in_maps


def run(support_features, query_features, trace=False, **trace_kwargs):
    nc = build()
    in_maps = _host_inputs(support_features, query_features)
    res = run_bass_kernel_spmd(nc, in_maps, list(range(NCORES)),
                               trace=trace, **trace_kwargs)
    outs = [np.asarray(r["out"]).reshape(QS) for r in res.results]
    return np.concatenate(outs).astype(np.int32), res


def kernel(support_features, query_features, use_cosine=None, **_ignored):
    # use_cosine does not change the result: with L2-normalized vectors the
    # euclidean argmin equals the cosine argmax (monotone map), so one kernel
    # serves both branches.
    out, _ = run(support_features, query_features, trace=False)
    return out
